# revision 14
# baseline (speedup 1.0000x reference)
"""Trainium2 Bass kernel for 16-head MHA (B=4, S=2048, HIDDEN=1024, fp32 io).

Sharding (8 NeuronCores): core c -> batch b = c//2, head-group g = c%2
(8 heads, 512 features each).  Tensor-parallel over heads within a batch:
q/k/v projections column-sharded, o_proj row-sharded; the two partial
o_proj outputs per batch are summed on the host (plus bo).

All matmul operands are bf16 (PSUM accumulation stays fp32): on TRN2
hardware fp32r matmuls run the LOW_HIGH double-pass, so bf16 halves
Tensor-engine stream time and weight-load time.  Matmul outputs are
capped at one PSUM bank (512 fp32), so every matmul runs N=512.

Layout strategy (per core):
  - x arrives pre-transposed (xT: [1024, 2048] bf16) and stays resident
    in SBUF ([128, 8, 2048], 32KB/partition) so projections can be
    emitted piecemeal while attention runs.
  - QT/KT [feature, seq] bf16 (feature on partitions), V natural
    [seq, feature] bf16 with a ones column (V2[..., 64] = 1) so the PV
    matmul accumulates the softmax denominator for free (row 64).
  - scores are computed transposed S.T[ks, qs] = KT.T @ QT with the two
    heads of a pair packed into the two 64-row PE row-tiles (concurrent
    matmuls, measured 99% overlap), written into one 2-bank PSUM tile so
    a single ScalarE exp covers both heads; exp writes bf16 directly.
  - PSUM (8 banks): scores pool 2x2 banks, PV accumulator [65, 1024]
    2 banks, and a dedicated double-buffered 1-bank "drip" pool through
    which V/K/Q projection tiles, o_proj tiles and the 1/Z broadcast
    flow without stalling each other.
  - Job scheduling: phase A only does K(pair0,slab0) + Q(pair0,slab0) +
    V(ss 0-3) (~25us); every remaining projection tile and all o_proj
    tiles are "drip jobs" emitted inside the attention ks-loops, one job
    per (or every other) key-chunk step, ordered so each tile completes
    just before its first consumer.  The PE never idles waiting for a
    phase boundary and exp starts ~50us earlier.
  - Deferred softmax normalization: unnormalized PV output is copied out
    immediately; 1/Z comes from a fast DVE reciprocal, converted
    fp32->bf16 on the idle GpSimd engine; TWO iterations later a K=1
    bf16 ones-matmul broadcasts 1/Z and an all-bf16 DVE multiply writes
    AOT.  The last two iterations convert on DVE instead and flush
    immediately to shorten the tail.
"""

import sys

if "/opt/trn_rl_repo" not in sys.path:
    sys.path.insert(0, "/opt/trn_rl_repo")

import numpy as np
import ml_dtypes

import concourse.tile as tile
from concourse import bacc, mybir
from concourse.bass_utils import run_bass_kernel_spmd

F32 = mybir.dt.float32
BF16 = mybir.dt.bfloat16
EXP = mybir.ActivationFunctionType.Exp
NP_BF16 = ml_dtypes.bfloat16

B, S, HID = 4, 2048, 1024
HEADS, D = 16, 64
NCORES = 8
O = HID // 2          # features per core (8 heads)
P = 128
KO = HID // P         # 8 contraction chunks for projections
NPAIR = 4             # head pairs per core
NQ = 4                # query blocks of 512
QB = S // NQ          # 512
NK = 16               # key chunks of 128
NSS = S // P          # 16 seq subtiles

_CACHE: dict = {}


def build_nc():
    nc = bacc.Bacc("TRN2", debug=False, target_bir_lowering=False,
                   num_devices=NCORES)

    xT = nc.dram_tensor("xT", [HID, S], BF16, kind="ExternalInput").ap()
    wqT = nc.dram_tensor("wqT", [HID, O], BF16, kind="ExternalInput").ap()
    wkT = nc.dram_tensor("wkT", [HID, O], BF16, kind="ExternalInput").ap()
    wvT = nc.dram_tensor("wvT", [HID, O], BF16, kind="ExternalInput").ap()
    woT = nc.dram_tensor("woT", [O, HID], BF16, kind="ExternalInput").ap()
    bq = nc.dram_tensor("bq", [P, NPAIR], F32, kind="ExternalInput").ap()
    bk = nc.dram_tensor("bk", [P, NPAIR], F32, kind="ExternalInput").ap()
    bv = nc.dram_tensor("bv", [1, O], F32, kind="ExternalInput").ap()
    y = nc.dram_tensor("y", [S, HID], F32, kind="ExternalOutput").ap()

    xT3 = xT.rearrange("(ko p) s -> p ko s", p=P)      # [128, 8, 2048]
    wqT3 = wqT.rearrange("(ko p) o -> p ko o", p=P)    # [128, 8, 512]
    wkT3 = wkT.rearrange("(ko p) o -> p ko o", p=P)
    wvT3 = wvT.rearrange("(ko p) o -> p ko o", p=P)
    woT3 = woT.rearrange("(oo p) j -> p oo j", p=P)    # [128, 4, 1024]

    with tile.TileContext(nc) as tc:
        # ---- long-lived SBUF tensors --------------------------------
        main_cm = tc.tile_pool(name="main", bufs=1)
        main = main_cm.__enter__()
        QT = main.tile([P, NPAIR, S], BF16, tag="QT")       # [128, 4, 2048]
        KT = main.tile([P, NPAIR, S], BF16, tag="KT")
        V2 = main.tile([P, NSS, 8, D + 1], BF16, tag="V2")  # [128, 16, 8, 65]
        XT = main.tile([P, KO, S], BF16, tag="XT")          # resident x
        ones_sb = main.tile([1, P], F32, tag="ones")
        ones_bf = main.tile([1, P], BF16, tag="onesbf")
        bq_sb = main.tile([P, NPAIR], F32, tag="bq")
        bk_sb = main.tile([P, NPAIR], F32, tag="bk")
        bv_sb = main.tile([1, O], F32, tag="bv")
        bvb_sb = main.tile([P, O], F32, tag="bvb")          # bv broadcast
        # projection weights outlive phase A (dripped into attention)
        wq_sb = main.tile([P, KO, O], BF16, tag="wq")
        wk_sb = main.tile([P, KO, O], BF16, tag="wk")
        wv_sb = main.tile([P, KO, O], BF16, tag="wv")

        nc.vector.memset(ones_sb[:], 1.0)
        nc.vector.memset(ones_bf[:], 1.0)
        nc.vector.memset(V2[:, :, :, D:D + 1], 1.0)

        # ---- projection job emitters (pool passed per phase) --------
        def jkq(pool, tag, which, pair, slab):
            w_sb, b_sb, dstT = {
                "k": (wk_sb, bk_sb, KT), "q": (wq_sb, bq_sb, QT)}[which]
            ps = pool.tile([P, QB], F32, tag=tag, name=f"ps_{which}")
            sl = slice(slab * QB, (slab + 1) * QB)
            for k in range(KO):
                nc.tensor.matmul(
                    ps[:], w_sb[:, k, pair * P:(pair + 1) * P],
                    XT[:, k, sl],
                    start=(k == 0), stop=(k == KO - 1))
            nc.vector.tensor_scalar_add(
                dstT[:, pair, sl], ps[:], b_sb[:, pair:pair + 1])

        def jv(pool, tag, ss):
            ps = pool.tile([P, QB], F32, tag=tag, name="ps_v")
            for k in range(KO):
                nc.tensor.matmul(
                    ps[:],
                    XT[:, k, ss * P:(ss + 1) * P],
                    wv_sb[:, k, :],
                    start=(k == 0), stop=(k == KO - 1))
            nc.vector.tensor_tensor(
                V2[:, ss, :, 0:D],
                ps.rearrange("p (h d) -> p h d", d=D),
                bvb_sb.rearrange("p (h d) -> p h d", d=D),
                mybir.AluOpType.add)

        # ---- phase A: minimal head start ----------------------------
        with tc.tile_pool(name="pa", bufs=3, space="PSUM") as ppa:
            nc.sync.dma_start(bv_sb[:], bv)
            nc.sync.dma_start(bk_sb[:], bk)
            nc.sync.dma_start(bq_sb[:], bq)
            for k in range(KO):
                nc.sync.dma_start(XT[:, k, 0:512], xT3[:, k, 0:512])
            for k in range(KO):
                nc.sync.dma_start(wk_sb[:, k, :], wkT3[:, k, :])
            for k in range(KO):
                nc.sync.dma_start(wv_sb[:, k, :], wvT3[:, k, :])
            for k in range(KO):
                nc.sync.dma_start(wq_sb[:, k, :], wqT3[:, k, :])
            for q4 in range(1, 4):
                for k in range(KO):
                    nc.sync.dma_start(
                        XT[:, k, q4 * 512:(q4 + 1) * 512],
                        xT3[:, k, q4 * 512:(q4 + 1) * 512])

            # broadcast bv across partitions with a K=1 ones-matmul
            ps_b = ppa.tile([P, QB], F32, tag="ps", name="ps_b")
            nc.tensor.matmul(ps_b[:], ones_sb[0:1, 0:P], bv_sb[0:1, :],
                             start=True, stop=True)
            nc.vector.tensor_copy(bvb_sb[:], ps_b[:])

            jkq(ppa, "ps", "k", 0, 0)
            jkq(ppa, "ps", "q", 0, 0)
            for ss in range(4):
                jv(ppa, "ps", ss)

        # ---- phase B: attention + dripped jobs ----------------------
        with tc.tile_pool(name="wo", bufs=1) as wopool, \
             tc.tile_pool(name="aot", bufs=1) as aotpool, \
             tc.tile_pool(name="pt", bufs=3) as ptpool, \
             tc.tile_pool(name="small", bufs=2) as spool, \
             tc.tile_pool(name="outsb", bufs=3) as opool, \
             tc.tile_pool(name="psc", bufs=2, space="PSUM") as psc, \
             tc.tile_pool(name="ppv", bufs=1, space="PSUM") as ppv, \
             tc.tile_pool(name="pdrip", bufs=2, space="PSUM") as pdrip:
            wo_sb = wopool.tile([P, NPAIR, HID], BF16, tag="wo")
            for oo in range(NPAIR):
                nc.sync.dma_start(wo_sb[:, oo, :], woT3[:, oo, :])
            AOT = aotpool.tile([P, NPAIR, S], BF16, tag="AOT")

            def jop(ss, jh):
                ps_o = pdrip.tile([P, QB], F32, tag="d", name="ps_o")
                for oo in range(NPAIR):
                    nc.tensor.matmul(
                        ps_o[:],
                        AOT[:, oo, ss * P:(ss + 1) * P],
                        wo_sb[:, oo, jh * QB:(jh + 1) * QB],
                        start=(oo == 0), stop=(oo == NPAIR - 1))
                ob = opool.tile([P, QB], F32, tag="ob", name="ob")
                nc.vector.tensor_copy(ob[:], ps_o[:])
                nc.sync.dma_start(
                    y[ss * P:(ss + 1) * P, jh * QB:(jh + 1) * QB], ob[:])

            drip_work = []

            def do_drip(n):
                for _ in range(n):
                    if not drip_work:
                        return
                    item = drip_work.pop(0)
                    if item[0] == "kq":
                        jkq(pdrip, "d", item[1], item[2], item[3])
                    elif item[0] == "v":
                        jv(pdrip, "d", item[1])
                    else:
                        jop(item[1], item[2])

            # deferred softmax normalization, stage B two iterations late
            pending = []

            def norm_stage_b(keep=0):
                while len(pending) > keep:
                    recip_bf, u_sb, aslc_ab = pending.pop(0)
                    for h in range(2):
                        bc_ps = pdrip.tile([P, QB], F32, tag="d",
                                           name="bc_ps")
                        nc.tensor.matmul(
                            bc_ps[0:D, :],
                            ones_bf[0:1, 0:D],
                            recip_bf[:, h * QB:(h + 1) * QB],
                            start=True, stop=True)
                        bc_sb = spool.tile([D, QB], BF16, tag=f"bc{h}",
                                           name="bc")
                        nc.vector.tensor_copy(bc_sb[:], bc_ps[0:D, :])
                        nc.vector.tensor_mul(
                            aslc_ab[h],
                            u_sb[:, h * QB:(h + 1) * QB],
                            bc_sb[:])

            for qi in range(NQ):
                qs = slice(qi * QB, (qi + 1) * QB)
                for pair in range(NPAIR):
                    it = qi * NPAIR + pair
                    # ---- job pushes whose deps are already met ------
                    if it == 0:
                        drip_work.extend(
                            [("kq", "k", 0, 1), ("v", 4), ("v", 5),
                             ("kq", "k", 0, 2), ("v", 6), ("v", 7),
                             ("v", 8), ("kq", "k", 0, 3)]
                            + [("v", ss) for ss in range(9, NSS)]
                            + [("kq", "k", 1, s) for s in range(NQ)]
                            + [("kq", "q", 1, 0)])
                    elif qi == 0 and pair < NPAIR - 1:
                        drip_work.extend(
                            [("kq", "k", pair + 1, s) for s in range(NQ)]
                            + [("kq", "q", pair + 1, 0)])
                    elif qi == 0:
                        drip_work.extend(
                            [("kq", "q", p, 1) for p in range(NPAIR)])
                    elif pair == 0 and qi < NQ - 1:
                        drip_work.extend(
                            [("kq", "q", p, qi + 1) for p in range(NPAIR)])

                    pv = ppv.tile([D + 1, 2 * QB], F32, tag="pv",
                                  name="pv")

                    def emit_pv(ks, pt):
                        for h in range(2):
                            nc.tensor.matmul(
                                pv[:, h * QB:(h + 1) * QB],
                                V2[:, ks, 2 * pair + h, :],
                                pt[:, h * QB:(h + 1) * QB],
                                start=(ks == 0), stop=(ks == NK - 1))

                    # PV is deferred one ks step so the next chunk's scores
                    # matmuls never sit behind a PV that waits on exp
                    prev_pv = None
                    for ks in range(NK):
                        sc = psc.tile([P, 2 * QB], F32, tag="sc", name="sc")
                        for h in range(2):
                            nc.tensor.matmul(
                                sc[:, h * QB:(h + 1) * QB],
                                KT[h * D:(h + 1) * D, pair,
                                   ks * P:(ks + 1) * P],
                                QT[h * D:(h + 1) * D, pair, qs],
                                start=True, stop=True)
                        pt = ptpool.tile([P, 2 * QB], BF16, tag="pt",
                                         name="pt")
                        nc.scalar.activation(pt[:], sc[:], EXP, scale=0.125)
                        if prev_pv is not None:
                            emit_pv(*prev_pv)
                        prev_pv = (ks, pt)
                        if it == 0 or ks % 2 == 0:
                            do_drip(1)
                    emit_pv(*prev_pv)
                    last2 = it >= NQ * NPAIR - 2
                    norm_stage_b(keep=0 if last2 else 1)
                    # stage A: Z -> 1/Z (DVE fp32), bf16 convert on GpSimd
                    # (DVE for the last two iterations -- shorter tail)
                    zrow = spool.tile([1, 2 * QB], F32, tag="zrow",
                                      name="zrow")
                    nc.vector.tensor_copy(zrow[:], pv[D:D + 1, :])
                    recip = spool.tile([1, 2 * QB], F32, tag="recip",
                                       name="recip")
                    nc.vector.reciprocal_approx_fast(recip[:], zrow[:])
                    recip_bf = spool.tile([1, 2 * QB], BF16, tag="recipbf",
                                          name="recipbf")
                    nc.vector.tensor_copy(recip_bf[:], recip[:])
                    u_sb = spool.tile([D, 2 * QB], BF16, tag="u", name="u")
                    nc.vector.tensor_copy(u_sb[:], pv[0:D, :])
                    pending.append(
                        (recip_bf, u_sb,
                         [AOT[h * D:(h + 1) * D, pair, qs]
                          for h in range(2)]))
                    # o_proj of block qi-1 becomes legal once the pending
                    # chain has flushed its pair-3 entry (two iterations)
                    if qi > 0 and pair == 1:
                        drip_work.extend(
                            [("op", ss, jh)
                             for ss in range((qi - 1) * NQ, qi * NQ)
                             for jh in range(2)])
                    do_drip(4)
            norm_stage_b()
            do_drip(len(drip_work))
            for ss in range((NQ - 1) * NQ, NSS):
                for jh in range(2):
                    jop(ss, jh)

        main_cm.__exit__(None, None, None)

    nc.compile()
    return nc


def prep_in_maps(x, Wq, bq, Wk, bk, Wv, bv, Wo, bo, head_mask):
    """Host-side shard + layout prep. Returns per-core input dicts."""
    xT = [np.ascontiguousarray(np.asarray(x[b]).T).astype(NP_BF16)
          for b in range(B)]
    per_group: dict = {}
    in_maps = []
    for c in range(NCORES):
        b, g = c // 2, c % 2
        rows = slice(g * O, (g + 1) * O)
        mask = np.repeat(np.asarray(head_mask[8 * g:8 * (g + 1)],
                                    dtype=np.float32), D)
        if g not in per_group:
            per_group[g] = {
                "wqT": np.ascontiguousarray(
                    np.asarray(Wq)[rows, :].T).astype(NP_BF16),
                "wkT": np.ascontiguousarray(
                    np.asarray(Wk)[rows, :].T).astype(NP_BF16),
                "wvT": np.ascontiguousarray(
                    np.asarray(Wv)[rows, :].T).astype(NP_BF16),
                "woT": np.ascontiguousarray(
                    np.asarray(Wo)[:, rows].T * mask[:, None]
                ).astype(NP_BF16),
                "bq": np.ascontiguousarray(
                    np.asarray(bq)[rows].reshape(NPAIR, P).T,
                    dtype=np.float32),
                "bk": np.ascontiguousarray(
                    np.asarray(bk)[rows].reshape(NPAIR, P).T,
                    dtype=np.float32),
                "bv": np.asarray(bv, dtype=np.float32)[rows].reshape(1, O),
            }
        m = dict(per_group[g])
        m["xT"] = xT[b]
        in_maps.append(m)
    return in_maps


def run(in_maps, trace=False):
    if "nc" not in _CACHE:
        _CACHE["nc"] = build_nc()
    return run_bass_kernel_spmd(_CACHE["nc"], in_maps, list(range(NCORES)),
                                trace=trace)


def kernel(x, Wq, bq, Wk, bk, Wv, bv, Wo, bo, head_mask):
    in_maps = prep_in_maps(x, Wq, bq, Wk, bk, Wv, bv, Wo, bo, head_mask)
    res = run(in_maps).results
    bo = np.asarray(bo, dtype=np.float32)
    out = np.empty((B, S, HID), dtype=np.float32)
    for b in range(B):
        out[b] = res[2 * b]["y"] + res[2 * b + 1]["y"] + bo
    return out


# revision 20
# speedup vs baseline: 1.2661x; 1.2661x over previous
"""Trainium2 Bass kernel for 16-head MHA (B=4, S=2048, HIDDEN=1024, fp32 io).

Sharding (8 NeuronCores): core c -> batch b = c//2, head-group g = c%2
(8 heads, 512 features each).  Tensor-parallel over heads within a batch:
q/k/v projections column-sharded, o_proj row-sharded; the two partial
o_proj outputs per batch are summed on the host (plus bo).

All matmul operands are bf16 (PSUM accumulation stays fp32): on TRN2
hardware fp32r matmuls run the LOW_HIGH double-pass, so bf16 halves
Tensor-engine stream time and weight-load time.  Matmul outputs are
capped at one PSUM bank (512 fp32), so every matmul runs N=512.

Layout strategy (per core):
  - x arrives pre-transposed (xT: [1024, 2048] bf16) and stays resident
    in SBUF ([128, 8, 2048], 32KB/partition) so projections can be
    emitted piecemeal while attention runs.
  - QT/KT [feature, seq] bf16 (feature on partitions), V natural
    [seq, feature] bf16 with a ones column (V2[..., 64] = 1) so the PV
    matmul accumulates the softmax denominator for free (row 64).
  - scores are computed transposed S.T[ks, qs] = KT.T @ QT with the two
    heads of a pair packed into the two 64-row PE row-tiles (concurrent
    matmuls, measured 99% overlap), written into one 2-bank PSUM tile so
    a single ScalarE exp covers both heads; exp writes bf16 directly.
  - PSUM (8 banks): scores pool 2x2 banks, PV accumulator [65, 1024]
    2 banks, and a dedicated double-buffered 1-bank "drip" pool through
    which V/K/Q projection tiles, o_proj tiles and the 1/Z broadcast
    flow without stalling each other.
  - Job scheduling: phase A only does K(pair0,slab0) + Q(pair0,slab0) +
    V(ss 0-3) (~25us); every remaining projection tile and all o_proj
    tiles are "drip jobs" emitted inside the attention ks-loops, one job
    per (or every other) key-chunk step, ordered so each tile completes
    just before its first consumer.  The PE never idles waiting for a
    phase boundary and exp starts ~50us earlier.
  - Deferred softmax normalization: unnormalized PV output is copied out
    immediately; 1/Z comes from a fast DVE reciprocal, converted
    fp32->bf16 on the idle GpSimd engine; TWO iterations later a K=1
    bf16 ones-matmul broadcasts 1/Z and an all-bf16 DVE multiply writes
    AOT.  The last two iterations convert on DVE instead and flush
    immediately to shorten the tail.
"""

import sys

if "/opt/trn_rl_repo" not in sys.path:
    sys.path.insert(0, "/opt/trn_rl_repo")

import numpy as np
import ml_dtypes

import concourse.tile as tile
from concourse import bacc, mybir
from concourse.bass_utils import run_bass_kernel_spmd

F32 = mybir.dt.float32
BF16 = mybir.dt.bfloat16
EXP = mybir.ActivationFunctionType.Exp
NP_BF16 = ml_dtypes.bfloat16

B, S, HID = 4, 2048, 1024
HEADS, D = 16, 64
NCORES = 8
O = HID // 2          # features per core (8 heads)
P = 128
KO = HID // P         # 8 contraction chunks for projections
NPAIR = 4             # head pairs per core
NQ = 4                # query blocks of 512
QB = S // NQ          # 512
NK = 16               # key chunks of 128
NSS = S // P          # 16 seq subtiles

_CACHE: dict = {}


def build_nc():
    nc = bacc.Bacc("TRN2", debug=False, target_bir_lowering=False,
                   num_devices=NCORES)

    xT = nc.dram_tensor("xT", [HID, S], BF16, kind="ExternalInput").ap()
    wqT = nc.dram_tensor("wqT", [HID, O], BF16, kind="ExternalInput").ap()
    wkT = nc.dram_tensor("wkT", [HID, O], BF16, kind="ExternalInput").ap()
    wvT = nc.dram_tensor("wvT", [HID, O], BF16, kind="ExternalInput").ap()
    woT = nc.dram_tensor("woT", [O, HID], BF16, kind="ExternalInput").ap()
    bq = nc.dram_tensor("bq", [P, NPAIR], F32, kind="ExternalInput").ap()
    bk = nc.dram_tensor("bk", [P, NPAIR], F32, kind="ExternalInput").ap()
    bv = nc.dram_tensor("bv", [1, O], F32, kind="ExternalInput").ap()
    y = nc.dram_tensor("y", [S, HID], F32, kind="ExternalOutput").ap()

    xT3 = xT.rearrange("(ko p) s -> p ko s", p=P)      # [128, 8, 2048]
    wqT3 = wqT.rearrange("(ko p) o -> p ko o", p=P)    # [128, 8, 512]
    wkT3 = wkT.rearrange("(ko p) o -> p ko o", p=P)
    wvT3 = wvT.rearrange("(ko p) o -> p ko o", p=P)
    woT3 = woT.rearrange("(oo p) j -> p oo j", p=P)    # [128, 4, 1024]

    with tile.TileContext(nc) as tc:
        # ---- long-lived SBUF tensors --------------------------------
        main_cm = tc.tile_pool(name="main", bufs=1)
        main = main_cm.__enter__()
        QT = main.tile([P, NPAIR, S], BF16, tag="QT")       # [128, 4, 2048]
        KT = main.tile([P, NPAIR, S], BF16, tag="KT")
        V2 = main.tile([P, NSS, 8, D + 1], BF16, tag="V2")  # [128, 16, 8, 65]
        XT = main.tile([P, KO, S], BF16, tag="XT")          # resident x
        ones_sb = main.tile([1, P], F32, tag="ones")
        ones_r = main.tile([1, P], mybir.dt.float32r, tag="onesr")
        ones_bf = main.tile([1, P], BF16, tag="onesbf")
        bq_sb = main.tile([P, NPAIR], F32, tag="bq")
        bk_sb = main.tile([P, NPAIR], F32, tag="bk")
        bv_sb = main.tile([1, O], F32, tag="bv")
        bvb_sb = main.tile([P, O], F32, tag="bvb")          # bv broadcast
        # projection weights outlive phase A (dripped into attention)
        wq_sb = main.tile([P, KO, O], BF16, tag="wq")
        wk_sb = main.tile([P, KO, O], BF16, tag="wk")
        wv_sb = main.tile([P, KO, O], BF16, tag="wv")

        nc.vector.memset(ones_sb[:], 1.0)
        nc.vector.memset(ones_r[:].bitcast(F32), 1.0)
        nc.vector.memset(ones_bf[:], 1.0)
        nc.vector.memset(V2[:, :, :, D:D + 1], 1.0)

        # ---- projection job emitters (pool passed per phase) --------
        def jkq(pool, tag, which, pair, slab):
            w_sb, b_sb, dstT = {
                "k": (wk_sb, bk_sb, KT), "q": (wq_sb, bq_sb, QT)}[which]
            ps = pool.tile([P, QB], F32, tag=tag, name=f"ps_{which}")
            sl = slice(slab * QB, (slab + 1) * QB)
            for k in range(KO):
                nc.tensor.matmul(
                    ps[:], w_sb[:, k, pair * P:(pair + 1) * P],
                    XT[:, k, sl],
                    start=(k == 0), stop=(k == KO - 1))
            nc.vector.tensor_scalar_add(
                dstT[:, pair, sl], ps[:], b_sb[:, pair:pair + 1])

        def jv(pool, tag, ss):
            ps = pool.tile([P, QB], F32, tag=tag, name="ps_v")
            for k in range(KO):
                nc.tensor.matmul(
                    ps[:],
                    XT[:, k, ss * P:(ss + 1) * P],
                    wv_sb[:, k, :],
                    start=(k == 0), stop=(k == KO - 1))
            nc.vector.tensor_tensor(
                V2[:, ss, :, 0:D],
                ps.rearrange("p (h d) -> p h d", d=D),
                bvb_sb.rearrange("p (h d) -> p h d", d=D),
                mybir.AluOpType.add)

        # ---- phase A: minimal head start ----------------------------
        with tc.tile_pool(name="pa", bufs=3, space="PSUM") as ppa:
            nc.sync.dma_start(bv_sb[:], bv)
            nc.sync.dma_start(bk_sb[:], bk)
            nc.sync.dma_start(bq_sb[:], bq)
            for k in range(KO):
                nc.sync.dma_start(XT[:, k, 0:512], xT3[:, k, 0:512])
            for k in range(KO):
                nc.sync.dma_start(wk_sb[:, k, :], wkT3[:, k, :])
            for k in range(KO):
                nc.sync.dma_start(wv_sb[:, k, :], wvT3[:, k, :])
            for k in range(KO):
                nc.sync.dma_start(wq_sb[:, k, :], wqT3[:, k, :])
            for q4 in range(1, 4):
                for k in range(KO):
                    nc.sync.dma_start(
                        XT[:, k, q4 * 512:(q4 + 1) * 512],
                        xT3[:, k, q4 * 512:(q4 + 1) * 512])

            # broadcast bv across partitions with a K=1 ones-matmul
            ps_b = ppa.tile([P, QB], F32, tag="ps", name="ps_b")
            nc.tensor.matmul(ps_b[:], ones_sb[0:1, 0:P], bv_sb[0:1, :],
                             start=True, stop=True)
            nc.vector.tensor_copy(bvb_sb[:], ps_b[:])

            jkq(ppa, "ps", "k", 0, 0)
            jkq(ppa, "ps", "q", 0, 0)
            for ss in range(4):
                jv(ppa, "ps", ss)

        # ---- phase B: attention + dripped jobs ----------------------
        with tc.tile_pool(name="wo", bufs=1) as wopool, \
             tc.tile_pool(name="aot", bufs=1) as aotpool, \
             tc.tile_pool(name="pt", bufs=3) as ptpool, \
             tc.tile_pool(name="small", bufs=2) as spool, \
             tc.tile_pool(name="outsb", bufs=3) as opool, \
             tc.tile_pool(name="psc", bufs=2, space="PSUM") as psc, \
             tc.tile_pool(name="ppv", bufs=1, space="PSUM") as ppv, \
             tc.tile_pool(name="pdrip", bufs=2, space="PSUM") as pdrip:
            wo_sb = wopool.tile([P, NPAIR, HID], BF16, tag="wo")
            for oo in range(NPAIR):
                nc.sync.dma_start(wo_sb[:, oo, :], woT3[:, oo, :])
            AOT = aotpool.tile([P, NPAIR, S], BF16, tag="AOT")

            def jop(ss, jh):
                ps_o = pdrip.tile([P, QB], F32, tag="d", name="ps_o")
                for oo in range(NPAIR):
                    nc.tensor.matmul(
                        ps_o[:],
                        AOT[:, oo, ss * P:(ss + 1) * P],
                        wo_sb[:, oo, jh * QB:(jh + 1) * QB],
                        start=(oo == 0), stop=(oo == NPAIR - 1))
                ob = opool.tile([P, QB], F32, tag="ob", name="ob")
                nc.vector.tensor_copy(ob[:], ps_o[:])
                nc.sync.dma_start(
                    y[ss * P:(ss + 1) * P, jh * QB:(jh + 1) * QB], ob[:])

            drip_work = []

            def do_drip(n):
                for _ in range(n):
                    if not drip_work:
                        return
                    item = drip_work.pop(0)
                    if item[0] == "kq":
                        jkq(pdrip, "d", item[1], item[2], item[3])
                    elif item[0] == "v":
                        jv(pdrip, "d", item[1])
                    else:
                        jop(item[1], item[2])

            # deferred softmax normalization: broadcast the raw Z row with
            # a K=1 fp32r ones-matmul, then take the fast reciprocal of the
            # broadcast (same DVE cost as a 1-row reciprocal) and multiply.
            # No GpSimd and no dtype-convert in the chain.
            pending = []

            def norm_stage_b(keep=0):
                while len(pending) > keep:
                    zrow, u_sb, aslc_ab = pending.pop(0)
                    for h in range(2):
                        bc_ps = pdrip.tile([P, QB], F32, tag="d",
                                           name="bc_ps")
                        nc.tensor.matmul(
                            bc_ps[0:D, :],
                            ones_r[0:1, 0:D],
                            zrow[:, h * QB:(h + 1) * QB],
                            start=True, stop=True)
                        bcr = spool.tile([D, QB], F32, tag=f"bcr{h}",
                                         name="bcr")
                        nc.vector.reciprocal_approx_fast(
                            bcr[:], bc_ps[0:D, :])
                        nc.vector.tensor_mul(
                            aslc_ab[h],
                            u_sb[:, h * QB:(h + 1) * QB],
                            bcr[:])

            for qi in range(NQ):
                qs = slice(qi * QB, (qi + 1) * QB)
                for pair in range(NPAIR):
                    it = qi * NPAIR + pair
                    # ---- job pushes whose deps are already met ------
                    if it == 0:
                        drip_work.extend(
                            [("kq", "k", 0, 1), ("v", 4), ("v", 5),
                             ("kq", "k", 0, 2), ("v", 6), ("v", 7),
                             ("v", 8), ("kq", "k", 0, 3)]
                            + [("v", ss) for ss in range(9, NSS)]
                            + [("kq", "k", 1, s) for s in range(NQ)]
                            + [("kq", "q", 1, 0)])
                    elif qi == 0 and pair < NPAIR - 1:
                        drip_work.extend(
                            [("kq", "k", pair + 1, s) for s in range(NQ)]
                            + [("kq", "q", pair + 1, 0)])
                    elif qi == 0:
                        drip_work.extend(
                            [("kq", "q", p, 1) for p in range(NPAIR)])
                    elif pair == 0 and qi < NQ - 1:
                        drip_work.extend(
                            [("kq", "q", p, qi + 1) for p in range(NPAIR)])

                    pv = ppv.tile([D + 1, 2 * QB], F32, tag="pv",
                                  name="pv")

                    def emit_pv(ks, pt):
                        for h in range(2):
                            nc.tensor.matmul(
                                pv[:, h * QB:(h + 1) * QB],
                                V2[:, ks, 2 * pair + h, :],
                                pt[:, h * QB:(h + 1) * QB],
                                start=(ks == 0), stop=(ks == NK - 1))

                    # PV is deferred one ks step so the next chunk's scores
                    # matmuls never sit behind a PV that waits on exp
                    prev_pv = None
                    for ks in range(NK):
                        sc = psc.tile([P, 2 * QB], F32, tag="sc", name="sc")
                        for h in range(2):
                            nc.tensor.matmul(
                                sc[:, h * QB:(h + 1) * QB],
                                KT[h * D:(h + 1) * D, pair,
                                   ks * P:(ks + 1) * P],
                                QT[h * D:(h + 1) * D, pair, qs],
                                start=True, stop=True)
                        pt = ptpool.tile([P, 2 * QB], BF16, tag="pt",
                                         name="pt")
                        nc.scalar.activation(pt[:], sc[:], EXP, scale=0.125)
                        if prev_pv is not None:
                            emit_pv(*prev_pv)
                        prev_pv = (ks, pt)
                        if it == 0 or ks % 2 == 0:
                            do_drip(1)
                    emit_pv(*prev_pv)
                    last2 = it >= NQ * NPAIR - 2
                    norm_stage_b(keep=0 if last2 else 1)
                    # stage A: copy out the Z row and the unnormalized
                    # values (frees the PSUM accumulator)
                    zrow = spool.tile([1, 2 * QB], mybir.dt.float32r,
                                      tag="zrow", name="zrow")
                    nc.vector.tensor_copy(zrow[:], pv[D:D + 1, :])
                    u_sb = spool.tile([D, 2 * QB], BF16, tag="u", name="u")
                    nc.vector.tensor_copy(u_sb[:], pv[0:D, :])
                    pending.append(
                        (zrow, u_sb,
                         [AOT[h * D:(h + 1) * D, pair, qs]
                          for h in range(2)]))
                    # o_proj of block qi-1 becomes legal once the pending
                    # chain has flushed its pair-3 entry (two iterations)
                    if qi > 0 and pair == 1:
                        drip_work.extend(
                            [("op", ss, jh)
                             for ss in range((qi - 1) * NQ, qi * NQ)
                             for jh in range(2)])
                    do_drip(4)
            norm_stage_b()
            do_drip(len(drip_work))
            for ss in range((NQ - 1) * NQ, NSS):
                for jh in range(2):
                    jop(ss, jh)

        main_cm.__exit__(None, None, None)

    nc.compile()
    return nc


def prep_in_maps(x, Wq, bq, Wk, bk, Wv, bv, Wo, bo, head_mask):
    """Host-side shard + layout prep. Returns per-core input dicts."""
    xT = [np.ascontiguousarray(np.asarray(x[b]).T).astype(NP_BF16)
          for b in range(B)]
    per_group: dict = {}
    in_maps = []
    for c in range(NCORES):
        b, g = c // 2, c % 2
        rows = slice(g * O, (g + 1) * O)
        mask = np.repeat(np.asarray(head_mask[8 * g:8 * (g + 1)],
                                    dtype=np.float32), D)
        if g not in per_group:
            per_group[g] = {
                "wqT": np.ascontiguousarray(
                    np.asarray(Wq)[rows, :].T).astype(NP_BF16),
                "wkT": np.ascontiguousarray(
                    np.asarray(Wk)[rows, :].T).astype(NP_BF16),
                "wvT": np.ascontiguousarray(
                    np.asarray(Wv)[rows, :].T).astype(NP_BF16),
                "woT": np.ascontiguousarray(
                    np.asarray(Wo)[:, rows].T * mask[:, None]
                ).astype(NP_BF16),
                "bq": np.ascontiguousarray(
                    np.asarray(bq)[rows].reshape(NPAIR, P).T,
                    dtype=np.float32),
                "bk": np.ascontiguousarray(
                    np.asarray(bk)[rows].reshape(NPAIR, P).T,
                    dtype=np.float32),
                "bv": np.asarray(bv, dtype=np.float32)[rows].reshape(1, O),
            }
        m = dict(per_group[g])
        m["xT"] = xT[b]
        in_maps.append(m)
    return in_maps


def run(in_maps, trace=False):
    if "nc" not in _CACHE:
        _CACHE["nc"] = build_nc()
    return run_bass_kernel_spmd(_CACHE["nc"], in_maps, list(range(NCORES)),
                                trace=trace)


def kernel(x, Wq, bq, Wk, bk, Wv, bv, Wo, bo, head_mask):
    in_maps = prep_in_maps(x, Wq, bq, Wk, bk, Wv, bv, Wo, bo, head_mask)
    res = run(in_maps).results
    bo = np.asarray(bo, dtype=np.float32)
    out = np.empty((B, S, HID), dtype=np.float32)
    for b in range(B):
        out[b] = res[2 * b]["y"] + res[2 * b + 1]["y"] + bo
    return out


# revision 22
# speedup vs baseline: 1.2769x; 1.0085x over previous
"""Trainium2 Bass kernel for 16-head MHA (B=4, S=2048, HIDDEN=1024, fp32 io).

Sharding (8 NeuronCores): core c -> batch b = c//2, head-group g = c%2
(8 heads, 512 features each).  Tensor-parallel over heads within a batch:
q/k/v projections column-sharded, o_proj row-sharded; the two partial
o_proj outputs per batch are summed on the host (plus bo).

All matmul operands are bf16 (PSUM accumulation stays fp32): on TRN2
hardware fp32r matmuls run the LOW_HIGH double-pass, so bf16 halves
Tensor-engine stream time and weight-load time.  Matmul outputs are
capped at one PSUM bank (512 fp32), so every matmul runs N=512.

Layout strategy (per core):
  - x arrives pre-transposed (xT: [1024, 2048] bf16) and stays resident
    in SBUF ([128, 8, 2048], 32KB/partition) so projections can be
    emitted piecemeal while attention runs.
  - QT/KT [feature, seq] bf16 (feature on partitions), V natural
    [seq, feature] bf16 with a ones column (V2[..., 64] = 1) so the PV
    matmul accumulates the softmax denominator for free (row 64).
  - scores are computed transposed S.T[ks, qs] = KT.T @ QT with the two
    heads of a pair packed into the two 64-row PE row-tiles (concurrent
    matmuls, measured 99% overlap), written into one 2-bank PSUM tile so
    a single ScalarE exp covers both heads; exp writes bf16 directly.
  - PSUM (8 banks): scores pool 2x2 banks, PV accumulator [65, 1024]
    2 banks, and a dedicated double-buffered 1-bank "drip" pool through
    which V/K/Q projection tiles, o_proj tiles and the 1/Z broadcast
    flow without stalling each other.
  - Job scheduling: phase A only does K(pair0,slab0) + Q(pair0,slab0) +
    V(ss 0-3) (~25us); every remaining projection tile and all o_proj
    tiles are "drip jobs" emitted inside the attention ks-loops, one job
    per (or every other) key-chunk step, ordered so each tile completes
    just before its first consumer.  The PE never idles waiting for a
    phase boundary and exp starts ~50us earlier.
  - Deferred softmax normalization: unnormalized PV output is copied out
    immediately; 1/Z comes from a fast DVE reciprocal, converted
    fp32->bf16 on the idle GpSimd engine; TWO iterations later a K=1
    bf16 ones-matmul broadcasts 1/Z and an all-bf16 DVE multiply writes
    AOT.  The last two iterations convert on DVE instead and flush
    immediately to shorten the tail.
"""

import sys

if "/opt/trn_rl_repo" not in sys.path:
    sys.path.insert(0, "/opt/trn_rl_repo")

import numpy as np
import ml_dtypes

import concourse.tile as tile
from concourse import bacc, mybir
from concourse.bass_utils import run_bass_kernel_spmd

F32 = mybir.dt.float32
BF16 = mybir.dt.bfloat16
EXP = mybir.ActivationFunctionType.Exp
NP_BF16 = ml_dtypes.bfloat16

B, S, HID = 4, 2048, 1024
HEADS, D = 16, 64
NCORES = 8
O = HID // 2          # features per core (8 heads)
P = 128
KO = HID // P         # 8 contraction chunks for projections
NPAIR = 4             # head pairs per core
NQ = 4                # query blocks of 512
QB = S // NQ          # 512
NK = 16               # key chunks of 128
NSS = S // P          # 16 seq subtiles

_CACHE: dict = {}


def build_nc():
    nc = bacc.Bacc("TRN2", debug=False, target_bir_lowering=False,
                   num_devices=NCORES)

    xT = nc.dram_tensor("xT", [HID, S], BF16, kind="ExternalInput").ap()
    wqT = nc.dram_tensor("wqT", [HID, O], BF16, kind="ExternalInput").ap()
    wkT = nc.dram_tensor("wkT", [HID, O], BF16, kind="ExternalInput").ap()
    wvT = nc.dram_tensor("wvT", [HID, O], BF16, kind="ExternalInput").ap()
    woT = nc.dram_tensor("woT", [O, HID], BF16, kind="ExternalInput").ap()
    bq = nc.dram_tensor("bq", [P, NPAIR], F32, kind="ExternalInput").ap()
    bk = nc.dram_tensor("bk", [P, NPAIR], F32, kind="ExternalInput").ap()
    bv = nc.dram_tensor("bv", [1, O], F32, kind="ExternalInput").ap()
    y = nc.dram_tensor("y", [S, HID], F32, kind="ExternalOutput").ap()

    xT3 = xT.rearrange("(ko p) s -> p ko s", p=P)      # [128, 8, 2048]
    wqT3 = wqT.rearrange("(ko p) o -> p ko o", p=P)    # [128, 8, 512]
    wkT3 = wkT.rearrange("(ko p) o -> p ko o", p=P)
    wvT3 = wvT.rearrange("(ko p) o -> p ko o", p=P)
    woT3 = woT.rearrange("(oo p) j -> p oo j", p=P)    # [128, 4, 1024]

    with tile.TileContext(nc) as tc:
        # ---- long-lived SBUF tensors --------------------------------
        main_cm = tc.tile_pool(name="main", bufs=1)
        main = main_cm.__enter__()
        QT = main.tile([P, NPAIR, S], BF16, tag="QT")       # [128, 4, 2048]
        KT = main.tile([P, NPAIR, S], BF16, tag="KT")
        V2 = main.tile([P, NSS, 8, D + 1], BF16, tag="V2")  # [128, 16, 8, 65]
        XT = main.tile([P, KO, S], BF16, tag="XT")          # resident x
        ones_sb = main.tile([1, P], F32, tag="ones")
        ones_r = main.tile([1, P], mybir.dt.float32r, tag="onesr")
        bq_sb = main.tile([P, NPAIR], F32, tag="bq")
        bk_sb = main.tile([P, NPAIR], F32, tag="bk")
        bv_sb = main.tile([1, O], F32, tag="bv")
        bvb_sb = main.tile([P, O], F32, tag="bvb")          # bv broadcast
        # projection weights outlive phase A (dripped into attention)
        wq_sb = main.tile([P, KO, O], BF16, tag="wq")
        wk_sb = main.tile([P, KO, O], BF16, tag="wk")
        wv_sb = main.tile([P, KO, O], BF16, tag="wv")

        nc.vector.memset(ones_sb[:], 1.0)
        nc.vector.memset(ones_r[:].bitcast(F32), 1.0)
        nc.vector.memset(V2[:, :, :, D:D + 1], 1.0)

        # ---- projection job emitters (pool passed per phase) --------
        def jkq(pool, tag, which, pair, slab):
            w_sb, b_sb, dstT = {
                "k": (wk_sb, bk_sb, KT), "q": (wq_sb, bq_sb, QT)}[which]
            ps = pool.tile([P, QB], F32, tag=tag, name=f"ps_{which}")
            sl = slice(slab * QB, (slab + 1) * QB)
            for k in range(KO):
                nc.tensor.matmul(
                    ps[:], w_sb[:, k, pair * P:(pair + 1) * P],
                    XT[:, k, sl],
                    start=(k == 0), stop=(k == KO - 1))
            nc.vector.tensor_scalar_add(
                dstT[:, pair, sl], ps[:], b_sb[:, pair:pair + 1])

        def jv(pool, tag, ss):
            ps = pool.tile([P, QB], F32, tag=tag, name="ps_v")
            for k in range(KO):
                nc.tensor.matmul(
                    ps[:],
                    XT[:, k, ss * P:(ss + 1) * P],
                    wv_sb[:, k, :],
                    start=(k == 0), stop=(k == KO - 1))
            nc.vector.tensor_tensor(
                V2[:, ss, :, 0:D],
                ps.rearrange("p (h d) -> p h d", d=D),
                bvb_sb.rearrange("p (h d) -> p h d", d=D),
                mybir.AluOpType.add)

        # ---- phase A: minimal head start ----------------------------
        with tc.tile_pool(name="pa", bufs=3, space="PSUM") as ppa:
            nc.sync.dma_start(bv_sb[:], bv)
            nc.sync.dma_start(bk_sb[:], bk)
            nc.sync.dma_start(bq_sb[:], bq)
            for k in range(KO):
                nc.sync.dma_start(wv_sb[:, k, :], wvT3[:, k, :])
            for k in range(KO):
                nc.sync.dma_start(XT[:, k, 0:512], xT3[:, k, 0:512])
            for k in range(KO):
                nc.sync.dma_start(wk_sb[:, k, :], wkT3[:, k, :])
            for k in range(KO):
                nc.sync.dma_start(wq_sb[:, k, :], wqT3[:, k, :])
            for q4 in range(1, 4):
                for k in range(KO):
                    nc.sync.dma_start(
                        XT[:, k, q4 * 512:(q4 + 1) * 512],
                        xT3[:, k, q4 * 512:(q4 + 1) * 512])

            # broadcast bv across partitions with a K=1 ones-matmul
            ps_b = ppa.tile([P, QB], F32, tag="ps", name="ps_b")
            nc.tensor.matmul(ps_b[:], ones_sb[0:1, 0:P], bv_sb[0:1, :],
                             start=True, stop=True)
            nc.vector.tensor_copy(bvb_sb[:], ps_b[:])

            for ss in range(4):
                jv(ppa, "ps", ss)
            jkq(ppa, "ps", "k", 0, 0)
            jkq(ppa, "ps", "q", 0, 0)

        # ---- phase B: attention + dripped jobs ----------------------
        with tc.tile_pool(name="wo", bufs=1) as wopool, \
             tc.tile_pool(name="aot", bufs=1) as aotpool, \
             tc.tile_pool(name="pt", bufs=3) as ptpool, \
             tc.tile_pool(name="small", bufs=2) as spool, \
             tc.tile_pool(name="outsb", bufs=3) as opool, \
             tc.tile_pool(name="psc", bufs=2, space="PSUM") as psc, \
             tc.tile_pool(name="ppv", bufs=1, space="PSUM") as ppv, \
             tc.tile_pool(name="pdrip", bufs=2, space="PSUM") as pdrip:
            wo_sb = wopool.tile([P, NPAIR, HID], BF16, tag="wo")
            for oo in range(NPAIR):
                nc.sync.dma_start(wo_sb[:, oo, :], woT3[:, oo, :])
            AOT = aotpool.tile([P, NPAIR, S], BF16, tag="AOT")

            def jop(ss, jh):
                ps_o = pdrip.tile([P, QB], F32, tag="d", name="ps_o")
                for oo in range(NPAIR):
                    nc.tensor.matmul(
                        ps_o[:],
                        AOT[:, oo, ss * P:(ss + 1) * P],
                        wo_sb[:, oo, jh * QB:(jh + 1) * QB],
                        start=(oo == 0), stop=(oo == NPAIR - 1))
                ob = opool.tile([P, QB], F32, tag="ob", name="ob")
                nc.vector.tensor_copy(ob[:], ps_o[:])
                nc.sync.dma_start(
                    y[ss * P:(ss + 1) * P, jh * QB:(jh + 1) * QB], ob[:])

            drip_work = []

            def do_drip(n):
                for _ in range(n):
                    if not drip_work:
                        return
                    item = drip_work.pop(0)
                    if item[0] == "kq":
                        jkq(pdrip, "d", item[1], item[2], item[3])
                    elif item[0] == "v":
                        jv(pdrip, "d", item[1])
                    else:
                        jop(item[1], item[2])

            # deferred softmax normalization: broadcast the raw Z row with
            # a K=1 fp32r ones-matmul, then take the fast reciprocal of the
            # broadcast (same DVE cost as a 1-row reciprocal) and multiply.
            # No GpSimd and no dtype-convert in the chain.
            pending = []

            def norm_stage_b(keep=0):
                while len(pending) > keep:
                    zrow, u_sb, aslc_ab = pending.pop(0)
                    for h in range(2):
                        bc_ps = pdrip.tile([P, QB], F32, tag="d",
                                           name="bc_ps")
                        nc.tensor.matmul(
                            bc_ps[0:D, :],
                            ones_r[0:1, 0:D],
                            zrow[:, h * QB:(h + 1) * QB],
                            start=True, stop=True)
                        bcr = spool.tile([D, QB], F32, tag=f"bcr{h}",
                                         name="bcr")
                        nc.vector.reciprocal_approx_fast(
                            bcr[:], bc_ps[0:D, :])
                        nc.vector.tensor_mul(
                            aslc_ab[h],
                            u_sb[:, h * QB:(h + 1) * QB],
                            bcr[:])

            for qi in range(NQ):
                qs = slice(qi * QB, (qi + 1) * QB)
                for pair in range(NPAIR):
                    it = qi * NPAIR + pair
                    # ---- job pushes whose deps are already met ------
                    if it == 0:
                        drip_work.extend(
                            [("kq", "k", 0, 1), ("v", 4), ("v", 5),
                             ("kq", "k", 0, 2), ("v", 6), ("v", 7),
                             ("v", 8), ("kq", "k", 0, 3)]
                            + [("v", ss) for ss in range(9, NSS)]
                            + [("kq", "k", 1, s) for s in range(NQ)]
                            + [("kq", "q", 1, 0)])
                    elif qi == 0 and pair < NPAIR - 1:
                        drip_work.extend(
                            [("kq", "k", pair + 1, s) for s in range(NQ)]
                            + [("kq", "q", pair + 1, 0)])
                    elif qi == 0:
                        drip_work.extend(
                            [("kq", "q", p, 1) for p in range(NPAIR)])
                    elif pair == 0 and qi < NQ - 1:
                        drip_work.extend(
                            [("kq", "q", p, qi + 1) for p in range(NPAIR)])

                    pv = ppv.tile([D + 1, 2 * QB], F32, tag="pv",
                                  name="pv")

                    def emit_pv(ks, pt):
                        for h in range(2):
                            nc.tensor.matmul(
                                pv[:, h * QB:(h + 1) * QB],
                                V2[:, ks, 2 * pair + h, :],
                                pt[:, h * QB:(h + 1) * QB],
                                start=(ks == 0), stop=(ks == NK - 1))

                    # PV is deferred one ks step so the next chunk's scores
                    # matmuls never sit behind a PV that waits on exp
                    prev_pv = None
                    for ks in range(NK):
                        sc = psc.tile([P, 2 * QB], F32, tag="sc", name="sc")
                        for h in range(2):
                            nc.tensor.matmul(
                                sc[:, h * QB:(h + 1) * QB],
                                KT[h * D:(h + 1) * D, pair,
                                   ks * P:(ks + 1) * P],
                                QT[h * D:(h + 1) * D, pair, qs],
                                start=True, stop=True)
                        pt = ptpool.tile([P, 2 * QB], BF16, tag="pt",
                                         name="pt")
                        nc.scalar.activation(pt[:], sc[:], EXP, scale=0.125)
                        if prev_pv is not None:
                            emit_pv(*prev_pv)
                        prev_pv = (ks, pt)
                        if it == 0 or ks % 2 == 0:
                            do_drip(1)
                    emit_pv(*prev_pv)
                    last2 = it >= NQ * NPAIR - 2
                    norm_stage_b(keep=0 if last2 else 1)
                    # stage A: copy out the Z row and the unnormalized
                    # values (frees the PSUM accumulator)
                    zrow = spool.tile([1, 2 * QB], mybir.dt.float32r,
                                      tag="zrow", name="zrow")
                    nc.vector.tensor_copy(zrow[:], pv[D:D + 1, :])
                    u_sb = spool.tile([D, 2 * QB], BF16, tag="u", name="u")
                    nc.vector.tensor_copy(u_sb[:], pv[0:D, :])
                    pending.append(
                        (zrow, u_sb,
                         [AOT[h * D:(h + 1) * D, pair, qs]
                          for h in range(2)]))
                    # o_proj of block qi-1 becomes legal once the pending
                    # chain has flushed its pair-3 entry (two iterations)
                    if qi > 0 and pair == 1:
                        drip_work.extend(
                            [("op", ss, jh)
                             for ss in range((qi - 1) * NQ, qi * NQ)
                             for jh in range(2)])
                    do_drip(4)
            norm_stage_b()
            do_drip(len(drip_work))
            for ss in range((NQ - 1) * NQ, NSS):
                for jh in range(2):
                    jop(ss, jh)

        main_cm.__exit__(None, None, None)

    nc.compile()
    return nc


def prep_in_maps(x, Wq, bq, Wk, bk, Wv, bv, Wo, bo, head_mask):
    """Host-side shard + layout prep. Returns per-core input dicts."""
    xT = [np.ascontiguousarray(np.asarray(x[b]).T).astype(NP_BF16)
          for b in range(B)]
    per_group: dict = {}
    in_maps = []
    for c in range(NCORES):
        b, g = c // 2, c % 2
        rows = slice(g * O, (g + 1) * O)
        mask = np.repeat(np.asarray(head_mask[8 * g:8 * (g + 1)],
                                    dtype=np.float32), D)
        if g not in per_group:
            per_group[g] = {
                "wqT": np.ascontiguousarray(
                    np.asarray(Wq)[rows, :].T).astype(NP_BF16),
                "wkT": np.ascontiguousarray(
                    np.asarray(Wk)[rows, :].T).astype(NP_BF16),
                "wvT": np.ascontiguousarray(
                    np.asarray(Wv)[rows, :].T).astype(NP_BF16),
                "woT": np.ascontiguousarray(
                    np.asarray(Wo)[:, rows].T * mask[:, None]
                ).astype(NP_BF16),
                "bq": np.ascontiguousarray(
                    np.asarray(bq)[rows].reshape(NPAIR, P).T,
                    dtype=np.float32),
                "bk": np.ascontiguousarray(
                    np.asarray(bk)[rows].reshape(NPAIR, P).T,
                    dtype=np.float32),
                "bv": np.asarray(bv, dtype=np.float32)[rows].reshape(1, O),
            }
        m = dict(per_group[g])
        m["xT"] = xT[b]
        in_maps.append(m)
    return in_maps


def run(in_maps, trace=False):
    if "nc" not in _CACHE:
        _CACHE["nc"] = build_nc()
    return run_bass_kernel_spmd(_CACHE["nc"], in_maps, list(range(NCORES)),
                                trace=trace)


def kernel(x, Wq, bq, Wk, bk, Wv, bv, Wo, bo, head_mask):
    in_maps = prep_in_maps(x, Wq, bq, Wk, bk, Wv, bv, Wo, bo, head_mask)
    res = run(in_maps).results
    bo = np.asarray(bo, dtype=np.float32)
    out = np.empty((B, S, HID), dtype=np.float32)
    for b in range(B):
        out[b] = res[2 * b]["y"] + res[2 * b + 1]["y"] + bo
    return out


# revision 27
# speedup vs baseline: 1.2877x; 1.0084x over previous
"""Trainium2 Bass kernel for 16-head MHA (B=4, S=2048, HIDDEN=1024, fp32 io).

Sharding (8 NeuronCores): core c -> batch b = c//2, head-group g = c%2
(8 heads, 512 features each).  Tensor-parallel over heads within a batch:
q/k/v projections column-sharded, o_proj row-sharded; the two partial
o_proj outputs per batch are summed on the host (plus bo).

All matmul operands are bf16 (PSUM accumulation stays fp32): on TRN2
hardware fp32r matmuls run the LOW_HIGH double-pass, so bf16 halves
Tensor-engine stream time and weight-load time.  Matmul outputs are
capped at one PSUM bank (512 fp32), so every matmul runs N=512.

Layout strategy (per core):
  - x arrives pre-transposed (xT: [1024, 2048] bf16) and stays resident
    in SBUF ([128, 8, 2048], 32KB/partition) so projections can be
    emitted piecemeal while attention runs.
  - QT/KT [feature, seq] bf16 (feature on partitions), V natural
    [seq, feature] bf16 with a ones column (V2[..., 64] = 1) so the PV
    matmul accumulates the softmax denominator for free (row 64).
  - scores are computed transposed S.T[ks, qs] = KT.T @ QT with the two
    heads of a pair packed into the two 64-row PE row-tiles (concurrent
    matmuls, measured 99% overlap), written into one 2-bank PSUM tile so
    a single ScalarE exp covers both heads; exp writes bf16 directly.
  - PSUM (8 banks): scores pool 2x2 banks, PV accumulator [65, 1024]
    2 banks, and a dedicated double-buffered 1-bank "drip" pool through
    which V/K/Q projection tiles, o_proj tiles and the 1/Z broadcast
    flow without stalling each other.
  - Job scheduling: phase A only does K(pair0,slab0) + Q(pair0,slab0) +
    V(ss 0-3) (~25us); every remaining projection tile and all o_proj
    tiles are "drip jobs" emitted inside the attention ks-loops, one job
    per (or every other) key-chunk step, ordered so each tile completes
    just before its first consumer.  The PE never idles waiting for a
    phase boundary and exp starts ~50us earlier.
  - Deferred softmax normalization: unnormalized PV output is copied out
    immediately; 1/Z comes from a fast DVE reciprocal, converted
    fp32->bf16 on the idle GpSimd engine; TWO iterations later a K=1
    bf16 ones-matmul broadcasts 1/Z and an all-bf16 DVE multiply writes
    AOT.  The last two iterations convert on DVE instead and flush
    immediately to shorten the tail.
"""

import sys

if "/opt/trn_rl_repo" not in sys.path:
    sys.path.insert(0, "/opt/trn_rl_repo")

import numpy as np
import ml_dtypes

import concourse.tile as tile
from concourse import bacc, mybir
from concourse.bass_utils import run_bass_kernel_spmd

F32 = mybir.dt.float32
BF16 = mybir.dt.bfloat16
EXP = mybir.ActivationFunctionType.Exp
NP_BF16 = ml_dtypes.bfloat16

B, S, HID = 4, 2048, 1024
HEADS, D = 16, 64
NCORES = 8
O = HID // 2          # features per core (8 heads)
P = 128
KO = HID // P         # 8 contraction chunks for projections
NPAIR = 4             # head pairs per core
NQ = 4                # query blocks of 512
QB = S // NQ          # 512
NK = 16               # key chunks of 128
NSS = S // P          # 16 seq subtiles

_CACHE: dict = {}


def build_nc():
    nc = bacc.Bacc("TRN2", debug=False, target_bir_lowering=False,
                   num_devices=NCORES)

    xT = nc.dram_tensor("xT", [HID, S], BF16, kind="ExternalInput").ap()
    wqT = nc.dram_tensor("wqT", [HID, O], BF16, kind="ExternalInput").ap()
    wkT = nc.dram_tensor("wkT", [HID, O], BF16, kind="ExternalInput").ap()
    wvT = nc.dram_tensor("wvT", [HID, O], BF16, kind="ExternalInput").ap()
    woT = nc.dram_tensor("woT", [O, HID], BF16, kind="ExternalInput").ap()
    bq = nc.dram_tensor("bq", [P, NPAIR], F32, kind="ExternalInput").ap()
    bk = nc.dram_tensor("bk", [P, NPAIR], F32, kind="ExternalInput").ap()
    bv = nc.dram_tensor("bv", [1, O], F32, kind="ExternalInput").ap()
    y = nc.dram_tensor("y", [S, HID], F32, kind="ExternalOutput").ap()

    xT3 = xT.rearrange("(ko p) s -> p ko s", p=P)      # [128, 8, 2048]
    wqT3 = wqT.rearrange("(ko p) o -> p ko o", p=P)    # [128, 8, 512]
    wkT3 = wkT.rearrange("(ko p) o -> p ko o", p=P)
    wvT3 = wvT.rearrange("(ko p) o -> p ko o", p=P)
    woT3 = woT.rearrange("(oo p) j -> p oo j", p=P)    # [128, 4, 1024]

    with tile.TileContext(nc) as tc:
        # ---- long-lived SBUF tensors --------------------------------
        main_cm = tc.tile_pool(name="main", bufs=1)
        main = main_cm.__enter__()
        QT = main.tile([P, NPAIR, S], BF16, tag="QT")       # [128, 4, 2048]
        KT = main.tile([P, NPAIR, S], BF16, tag="KT")
        V2 = main.tile([P, NSS, 8, D + 1], BF16, tag="V2")  # [128, 16, 8, 65]
        XT = main.tile([P, KO, S], BF16, tag="XT")          # resident x
        ones_sb = main.tile([1, P], F32, tag="ones")
        ones_r = main.tile([1, P], mybir.dt.float32r, tag="onesr")
        bq_sb = main.tile([P, NPAIR], F32, tag="bq")
        bk_sb = main.tile([P, NPAIR], F32, tag="bk")
        bv_sb = main.tile([1, O], F32, tag="bv")
        bvb_sb = main.tile([P, O], F32, tag="bvb")          # bv broadcast
        # projection weights outlive phase A (dripped into attention)
        wq_sb = main.tile([P, KO, O], BF16, tag="wq")
        wk_sb = main.tile([P, KO, O], BF16, tag="wk")
        wv_sb = main.tile([P, KO, O], BF16, tag="wv")

        nc.vector.memset(ones_sb[:], 1.0)
        nc.vector.memset(ones_r[:].bitcast(F32), 1.0)
        nc.vector.memset(V2[:, :, :, D:D + 1], 1.0)

        # ---- projection job emitters (pool passed per phase) --------
        def jkq(pool, tag, which, pair, slab):
            w_sb, b_sb, dstT = {
                "k": (wk_sb, bk_sb, KT), "q": (wq_sb, bq_sb, QT)}[which]
            ps = pool.tile([P, QB], F32, tag=tag, name=f"ps_{which}")
            sl = slice(slab * QB, (slab + 1) * QB)
            for k in range(KO):
                nc.tensor.matmul(
                    ps[:], w_sb[:, k, pair * P:(pair + 1) * P],
                    XT[:, k, sl],
                    start=(k == 0), stop=(k == KO - 1))
            nc.vector.tensor_scalar_add(
                dstT[:, pair, sl], ps[:], b_sb[:, pair:pair + 1])

        def jv(pool, tag, ss):
            ps = pool.tile([P, QB], F32, tag=tag, name="ps_v")
            for k in range(KO):
                nc.tensor.matmul(
                    ps[:],
                    XT[:, k, ss * P:(ss + 1) * P],
                    wv_sb[:, k, :],
                    start=(k == 0), stop=(k == KO - 1))
            nc.vector.tensor_tensor(
                V2[:, ss, :, 0:D],
                ps.rearrange("p (h d) -> p h d", d=D),
                bvb_sb.rearrange("p (h d) -> p h d", d=D),
                mybir.AluOpType.add)

        # ---- phase A: minimal head start ----------------------------
        with tc.tile_pool(name="pa", bufs=3, space="PSUM") as ppa:
            nc.sync.dma_start(bv_sb[:], bv)
            nc.sync.dma_start(bk_sb[:], bk)
            nc.sync.dma_start(bq_sb[:], bq)
            for k in range(KO):
                nc.sync.dma_start(wv_sb[:, k, :], wvT3[:, k, :])
            for k in range(KO):
                nc.sync.dma_start(XT[:, k, 0:512], xT3[:, k, 0:512])
            for k in range(KO):
                nc.sync.dma_start(wk_sb[:, k, :], wkT3[:, k, :])
            for k in range(KO):
                nc.sync.dma_start(wq_sb[:, k, :], wqT3[:, k, :])
            for q4 in range(1, 4):
                for k in range(KO):
                    nc.sync.dma_start(
                        XT[:, k, q4 * 512:(q4 + 1) * 512],
                        xT3[:, k, q4 * 512:(q4 + 1) * 512])

            # broadcast bv across partitions with a K=1 ones-matmul
            ps_b = ppa.tile([P, QB], F32, tag="ps", name="ps_b")
            nc.tensor.matmul(ps_b[:], ones_sb[0:1, 0:P], bv_sb[0:1, :],
                             start=True, stop=True)
            nc.vector.tensor_copy(bvb_sb[:], ps_b[:])

            for ss in range(4):
                jv(ppa, "ps", ss)
            jkq(ppa, "ps", "k", 0, 0)
            jkq(ppa, "ps", "q", 0, 0)

        # ---- phase B: attention + dripped jobs ----------------------
        with tc.tile_pool(name="wo", bufs=1) as wopool, \
             tc.tile_pool(name="aot", bufs=1) as aotpool, \
             tc.tile_pool(name="pt", bufs=3) as ptpool, \
             tc.tile_pool(name="small", bufs=2) as spool, \
             tc.tile_pool(name="outsb", bufs=3) as opool, \
             tc.tile_pool(name="psc", bufs=2, space="PSUM") as psc, \
             tc.tile_pool(name="ppv", bufs=1, space="PSUM") as ppv, \
             tc.tile_pool(name="pdrip", bufs=2, space="PSUM") as pdrip:
            wo_sb = wopool.tile([P, NPAIR, HID], BF16, tag="wo")
            for oo in range(NPAIR):
                nc.sync.dma_start(wo_sb[:, oo, :], woT3[:, oo, :])
            AOT = aotpool.tile([P, NPAIR, S], BF16, tag="AOT")

            def jop(ss, jh, ps_o=None):
                if ps_o is None:
                    ps_o = pdrip.tile([P, QB], F32, tag="d", name="ps_o")
                for oo in range(NPAIR):
                    nc.tensor.matmul(
                        ps_o[:],
                        AOT[:, oo, ss * P:(ss + 1) * P],
                        wo_sb[:, oo, jh * QB:(jh + 1) * QB],
                        start=(oo == 0), stop=(oo == NPAIR - 1))
                ob = opool.tile([P, QB], F32, tag="ob", name="ob")
                nc.vector.tensor_copy(ob[:], ps_o[:])
                nc.sync.dma_start(
                    y[ss * P:(ss + 1) * P, jh * QB:(jh + 1) * QB], ob[:])

            drip_work = []

            def do_drip(n):
                for _ in range(n):
                    if not drip_work:
                        return
                    item = drip_work.pop(0)
                    if item[0] == "kq":
                        jkq(pdrip, "d", item[1], item[2], item[3])
                    elif item[0] == "v":
                        jv(pdrip, "d", item[1])
                    else:
                        jop(item[1], item[2])

            # deferred softmax normalization: broadcast the raw Z row with
            # a K=1 fp32r ones-matmul, then take the fast reciprocal of the
            # broadcast (same DVE cost as a 1-row reciprocal) and multiply.
            # No GpSimd and no dtype-convert in the chain.
            pending = []

            def norm_stage_b(keep=0):
                while len(pending) > keep:
                    zrow, u_sb, aslc_ab = pending.pop(0)
                    for h in range(2):
                        bc_ps = pdrip.tile([P, QB], F32, tag="d",
                                           name="bc_ps")
                        nc.tensor.matmul(
                            bc_ps[0:D, :],
                            ones_r[0:1, 0:D],
                            zrow[:, h * QB:(h + 1) * QB],
                            start=True, stop=True)
                        bcr = spool.tile([D, QB], F32, tag=f"bcr{h}",
                                         name="bcr")
                        nc.vector.reciprocal_approx_fast(
                            bcr[:], bc_ps[0:D, :])
                        nc.vector.tensor_mul(
                            aslc_ab[h],
                            u_sb[:, h * QB:(h + 1) * QB],
                            bcr[:])

            for qi in range(NQ):
                qs = slice(qi * QB, (qi + 1) * QB)
                for pair in range(NPAIR):
                    it = qi * NPAIR + pair
                    # ---- job pushes whose deps are already met ------
                    if it == 0:
                        drip_work.extend(
                            [("kq", "k", 0, 1), ("v", 4), ("v", 5),
                             ("kq", "k", 0, 2), ("v", 6), ("v", 7),
                             ("v", 8), ("kq", "k", 0, 3)]
                            + [("v", ss) for ss in range(9, NSS)]
                            + [("kq", "k", 1, s) for s in range(NQ)]
                            + [("kq", "q", 1, 0)])
                    elif qi == 0 and pair < NPAIR - 1:
                        drip_work.extend(
                            [("kq", "k", pair + 1, s) for s in range(NQ)]
                            + [("kq", "q", pair + 1, 0)])
                    elif qi == 0:
                        drip_work.extend(
                            [("kq", "q", p, 1) for p in range(NPAIR)])
                    elif pair == 0 and qi < NQ - 1:
                        drip_work.extend(
                            [("kq", "q", p, qi + 1) for p in range(NPAIR)])

                    pv = ppv.tile([D + 1, 2 * QB], F32, tag="pv",
                                  name="pv")

                    def emit_pv(ks, pt):
                        for h in range(2):
                            nc.tensor.matmul(
                                pv[:, h * QB:(h + 1) * QB],
                                V2[:, ks, 2 * pair + h, :],
                                pt[:, h * QB:(h + 1) * QB],
                                start=(ks == 0), stop=(ks == NK - 1))

                    # PV is deferred one ks step so the next chunk's scores
                    # matmuls never sit behind a PV that waits on exp
                    prev_pv = None
                    for ks in range(NK):
                        sc = psc.tile([P, 2 * QB], F32, tag="sc", name="sc")
                        for h in range(2):
                            nc.tensor.matmul(
                                sc[:, h * QB:(h + 1) * QB],
                                KT[h * D:(h + 1) * D, pair,
                                   ks * P:(ks + 1) * P],
                                QT[h * D:(h + 1) * D, pair, qs],
                                start=True, stop=True)
                        pt = ptpool.tile([P, 2 * QB], BF16, tag="pt",
                                         name="pt")
                        nc.scalar.activation(pt[:], sc[:], EXP, scale=0.125)
                        if prev_pv is not None:
                            emit_pv(*prev_pv)
                        prev_pv = (ks, pt)
                        if it == 0 or ks % 2 == 0:
                            do_drip(1)
                    emit_pv(*prev_pv)
                    last2 = it >= NQ * NPAIR - 2
                    norm_stage_b(keep=0 if last2 else 1)
                    # stage A: copy out the Z row and the unnormalized
                    # values (frees the PSUM accumulator)
                    zrow = spool.tile([1, 2 * QB], mybir.dt.float32r,
                                      tag="zrow", name="zrow")
                    nc.vector.tensor_copy(zrow[:], pv[D:D + 1, :])
                    u_sb = spool.tile([D, 2 * QB], BF16, tag="u", name="u")
                    nc.vector.tensor_copy(u_sb[:], pv[0:D, :])
                    pending.append(
                        (zrow, u_sb,
                         [AOT[h * D:(h + 1) * D, pair, qs]
                          for h in range(2)]))
                    # o_proj of block qi-1 becomes legal once the pending
                    # chain has flushed its pair-3 entry (two iterations)
                    if qi > 0 and pair == 1:
                        drip_work.extend(
                            [("op", ss, jh)
                             for ss in range((qi - 1) * NQ, qi * NQ)
                             for jh in range(2)])
                    do_drip(4)
            norm_stage_b()
            do_drip(len(drip_work))
            # tail o_proj: the scores pool is idle now -- alternate between
            # it and the drip pool for a 4-buffer pipeline
            for i, (ss, jh) in enumerate(
                    [(ss, jh) for ss in range((NQ - 1) * NQ, NSS)
                     for jh in range(2)]):
                if i % 2 == 0:
                    jop(ss, jh, psc.tile([P, 2 * QB], F32, tag="sc",
                                         name="ps_o")[:, 0:QB])
                else:
                    jop(ss, jh)

        main_cm.__exit__(None, None, None)

    nc.compile()
    return nc


def prep_in_maps(x, Wq, bq, Wk, bk, Wv, bv, Wo, bo, head_mask):
    """Host-side shard + layout prep. Returns per-core input dicts."""
    xT = [np.ascontiguousarray(np.asarray(x[b]).T).astype(NP_BF16)
          for b in range(B)]
    per_group: dict = {}
    in_maps = []
    for c in range(NCORES):
        b, g = c // 2, c % 2
        rows = slice(g * O, (g + 1) * O)
        mask = np.repeat(np.asarray(head_mask[8 * g:8 * (g + 1)],
                                    dtype=np.float32), D)
        if g not in per_group:
            per_group[g] = {
                "wqT": np.ascontiguousarray(
                    np.asarray(Wq)[rows, :].T).astype(NP_BF16),
                "wkT": np.ascontiguousarray(
                    np.asarray(Wk)[rows, :].T).astype(NP_BF16),
                "wvT": np.ascontiguousarray(
                    np.asarray(Wv)[rows, :].T).astype(NP_BF16),
                "woT": np.ascontiguousarray(
                    np.asarray(Wo)[:, rows].T * mask[:, None]
                ).astype(NP_BF16),
                "bq": np.ascontiguousarray(
                    np.asarray(bq)[rows].reshape(NPAIR, P).T,
                    dtype=np.float32),
                "bk": np.ascontiguousarray(
                    np.asarray(bk)[rows].reshape(NPAIR, P).T,
                    dtype=np.float32),
                "bv": np.asarray(bv, dtype=np.float32)[rows].reshape(1, O),
            }
        m = dict(per_group[g])
        m["xT"] = xT[b]
        in_maps.append(m)
    return in_maps


def run(in_maps, trace=False):
    if "nc" not in _CACHE:
        _CACHE["nc"] = build_nc()
    return run_bass_kernel_spmd(_CACHE["nc"], in_maps, list(range(NCORES)),
                                trace=trace)


def kernel(x, Wq, bq, Wk, bk, Wv, bv, Wo, bo, head_mask):
    in_maps = prep_in_maps(x, Wq, bq, Wk, bk, Wv, bv, Wo, bo, head_mask)
    res = run(in_maps).results
    bo = np.asarray(bo, dtype=np.float32)
    out = np.empty((B, S, HID), dtype=np.float32)
    for b in range(B):
        out[b] = res[2 * b]["y"] + res[2 * b + 1]["y"] + bo
    return out


# revision 28
# speedup vs baseline: 1.2987x; 1.0086x over previous
"""Trainium2 Bass kernel for 16-head MHA (B=4, S=2048, HIDDEN=1024, fp32 io).

Sharding (8 NeuronCores): core c -> batch b = c//2, head-group g = c%2
(8 heads, 512 features each).  Tensor-parallel over heads within a batch:
q/k/v projections column-sharded, o_proj row-sharded; the two partial
o_proj outputs per batch are summed on the host (plus bo).

All matmul operands are bf16 (PSUM accumulation stays fp32): on TRN2
hardware fp32r matmuls run the LOW_HIGH double-pass, so bf16 halves
Tensor-engine stream time and weight-load time.  Matmul outputs are
capped at one PSUM bank (512 fp32), so every matmul runs N=512.

Layout strategy (per core):
  - x arrives pre-transposed (xT: [1024, 2048] bf16) and stays resident
    in SBUF ([128, 8, 2048], 32KB/partition) so projections can be
    emitted piecemeal while attention runs.
  - QT/KT [feature, seq] bf16 (feature on partitions), V natural
    [seq, feature] bf16 with a ones column (V2[..., 64] = 1) so the PV
    matmul accumulates the softmax denominator for free (row 64).
  - scores are computed transposed S.T[ks, qs] = KT.T @ QT with the two
    heads of a pair packed into the two 64-row PE row-tiles (concurrent
    matmuls, measured 99% overlap), written into one 2-bank PSUM tile so
    a single ScalarE exp covers both heads; exp writes bf16 directly.
  - PSUM (8 banks): scores pool 2x2 banks, PV accumulator [65, 1024]
    2 banks, and a dedicated double-buffered 1-bank "drip" pool through
    which V/K/Q projection tiles, o_proj tiles and the 1/Z broadcast
    flow without stalling each other.
  - Job scheduling: phase A only does K(pair0,slab0) + Q(pair0,slab0) +
    V(ss 0-3) (~25us); every remaining projection tile and all o_proj
    tiles are "drip jobs" emitted inside the attention ks-loops, one job
    per (or every other) key-chunk step, ordered so each tile completes
    just before its first consumer.  The PE never idles waiting for a
    phase boundary and exp starts ~50us earlier.
  - Deferred softmax normalization: unnormalized PV output is copied out
    immediately; 1/Z comes from a fast DVE reciprocal, converted
    fp32->bf16 on the idle GpSimd engine; TWO iterations later a K=1
    bf16 ones-matmul broadcasts 1/Z and an all-bf16 DVE multiply writes
    AOT.  The last two iterations convert on DVE instead and flush
    immediately to shorten the tail.
"""

import sys

if "/opt/trn_rl_repo" not in sys.path:
    sys.path.insert(0, "/opt/trn_rl_repo")

import numpy as np
import ml_dtypes

import concourse.tile as tile
from concourse import bacc, mybir
from concourse.bass_utils import run_bass_kernel_spmd

F32 = mybir.dt.float32
BF16 = mybir.dt.bfloat16
EXP = mybir.ActivationFunctionType.Exp
NP_BF16 = ml_dtypes.bfloat16

B, S, HID = 4, 2048, 1024
HEADS, D = 16, 64
NCORES = 8
O = HID // 2          # features per core (8 heads)
P = 128
KO = HID // P         # 8 contraction chunks for projections
NPAIR = 4             # head pairs per core
NQ = 4                # query blocks of 512
QB = S // NQ          # 512
NK = 16               # key chunks of 128
NSS = S // P          # 16 seq subtiles

_CACHE: dict = {}


def build_nc():
    nc = bacc.Bacc("TRN2", debug=False, target_bir_lowering=False,
                   num_devices=NCORES)

    xT = nc.dram_tensor("xT", [HID, S], BF16, kind="ExternalInput").ap()
    wqT = nc.dram_tensor("wqT", [HID, O], BF16, kind="ExternalInput").ap()
    wkT = nc.dram_tensor("wkT", [HID, O], BF16, kind="ExternalInput").ap()
    wvT = nc.dram_tensor("wvT", [HID, O], BF16, kind="ExternalInput").ap()
    woT = nc.dram_tensor("woT", [O, HID], BF16, kind="ExternalInput").ap()
    bq = nc.dram_tensor("bq", [P, NPAIR], F32, kind="ExternalInput").ap()
    bk = nc.dram_tensor("bk", [P, NPAIR], F32, kind="ExternalInput").ap()
    bv = nc.dram_tensor("bv", [1, O], F32, kind="ExternalInput").ap()
    y = nc.dram_tensor("y", [S, HID], F32, kind="ExternalOutput").ap()

    xT3 = xT.rearrange("(ko p) s -> p ko s", p=P)      # [128, 8, 2048]
    wqT3 = wqT.rearrange("(ko p) o -> p ko o", p=P)    # [128, 8, 512]
    wkT3 = wkT.rearrange("(ko p) o -> p ko o", p=P)
    wvT3 = wvT.rearrange("(ko p) o -> p ko o", p=P)
    woT3 = woT.rearrange("(oo p) j -> p oo j", p=P)    # [128, 4, 1024]

    with tile.TileContext(nc) as tc:
        # ---- long-lived SBUF tensors --------------------------------
        main_cm = tc.tile_pool(name="main", bufs=1)
        main = main_cm.__enter__()
        QT = main.tile([P, NPAIR, S], BF16, tag="QT")       # [128, 4, 2048]
        KT = main.tile([P, NPAIR, S], BF16, tag="KT")
        V2 = main.tile([P, NSS, 8, D + 1], BF16, tag="V2")  # [128, 16, 8, 65]
        XT = main.tile([P, KO, S], BF16, tag="XT")          # resident x
        ones_sb = main.tile([1, P], F32, tag="ones")
        ones_bf = main.tile([1, P], BF16, tag="onesbf")
        bq_sb = main.tile([P, NPAIR], F32, tag="bq")
        bk_sb = main.tile([P, NPAIR], F32, tag="bk")
        bv_sb = main.tile([1, O], F32, tag="bv")
        bvb_sb = main.tile([P, O], F32, tag="bvb")          # bv broadcast
        # projection weights outlive phase A (dripped into attention)
        wq_sb = main.tile([P, KO, O], BF16, tag="wq")
        wk_sb = main.tile([P, KO, O], BF16, tag="wk")
        wv_sb = main.tile([P, KO, O], BF16, tag="wv")

        nc.vector.memset(ones_sb[:], 1.0)
        nc.vector.memset(ones_bf[:], 1.0)
        nc.vector.memset(V2[:, :, :, D:D + 1], 1.0)

        # ---- projection job emitters (pool passed per phase) --------
        def jkq(pool, tag, which, pair, slab):
            w_sb, b_sb, dstT = {
                "k": (wk_sb, bk_sb, KT), "q": (wq_sb, bq_sb, QT)}[which]
            ps = pool.tile([P, QB], F32, tag=tag, name=f"ps_{which}")
            sl = slice(slab * QB, (slab + 1) * QB)
            for k in range(KO):
                nc.tensor.matmul(
                    ps[:], w_sb[:, k, pair * P:(pair + 1) * P],
                    XT[:, k, sl],
                    start=(k == 0), stop=(k == KO - 1))
            nc.vector.tensor_scalar_add(
                dstT[:, pair, sl], ps[:], b_sb[:, pair:pair + 1])

        def jv(pool, tag, ss):
            ps = pool.tile([P, QB], F32, tag=tag, name="ps_v")
            for k in range(KO):
                nc.tensor.matmul(
                    ps[:],
                    XT[:, k, ss * P:(ss + 1) * P],
                    wv_sb[:, k, :],
                    start=(k == 0), stop=(k == KO - 1))
            nc.vector.tensor_tensor(
                V2[:, ss, :, 0:D],
                ps.rearrange("p (h d) -> p h d", d=D),
                bvb_sb.rearrange("p (h d) -> p h d", d=D),
                mybir.AluOpType.add)

        # ---- phase A: minimal head start ----------------------------
        with tc.tile_pool(name="pa", bufs=3, space="PSUM") as ppa:
            nc.sync.dma_start(bv_sb[:], bv)
            nc.sync.dma_start(bk_sb[:], bk)
            nc.sync.dma_start(bq_sb[:], bq)
            # whole-tensor DMAs: each dma_start costs the SP ~565ns of
            # sequencer time, so fewer/bigger transfers start compute sooner
            nc.sync.dma_start(wv_sb[:], wvT3[:, :, :])
            nc.sync.dma_start(XT[:, :, 0:512], xT3[:, :, 0:512])
            nc.sync.dma_start(wk_sb[:], wkT3[:, :, :])
            nc.sync.dma_start(wq_sb[:], wqT3[:, :, :])
            nc.sync.dma_start(XT[:, :, 512:S], xT3[:, :, 512:S])

            # broadcast bv across partitions with a K=1 ones-matmul
            ps_b = ppa.tile([P, QB], F32, tag="ps", name="ps_b")
            nc.tensor.matmul(ps_b[:], ones_sb[0:1, 0:P], bv_sb[0:1, :],
                             start=True, stop=True)
            nc.vector.tensor_copy(bvb_sb[:], ps_b[:])

            for ss in range(4):
                jv(ppa, "ps", ss)
            jkq(ppa, "ps", "k", 0, 0)
            jkq(ppa, "ps", "q", 0, 0)

        # ---- phase B: attention + dripped jobs ----------------------
        with tc.tile_pool(name="wo", bufs=1) as wopool, \
             tc.tile_pool(name="aot", bufs=1) as aotpool, \
             tc.tile_pool(name="pt", bufs=3) as ptpool, \
             tc.tile_pool(name="small", bufs=2) as spool, \
             tc.tile_pool(name="outsb", bufs=3) as opool, \
             tc.tile_pool(name="psc", bufs=2, space="PSUM") as psc, \
             tc.tile_pool(name="ppv", bufs=1, space="PSUM") as ppv, \
             tc.tile_pool(name="pdrip", bufs=2, space="PSUM") as pdrip:
            wo_sb = wopool.tile([P, NPAIR, HID], BF16, tag="wo")
            nc.sync.dma_start(wo_sb[:], woT3[:, :, :])
            AOT = aotpool.tile([P, NPAIR, S], BF16, tag="AOT")

            def jop(ss, jh, ps_o=None):
                if ps_o is None:
                    ps_o = pdrip.tile([P, QB], F32, tag="d", name="ps_o")
                for oo in range(NPAIR):
                    nc.tensor.matmul(
                        ps_o[:],
                        AOT[:, oo, ss * P:(ss + 1) * P],
                        wo_sb[:, oo, jh * QB:(jh + 1) * QB],
                        start=(oo == 0), stop=(oo == NPAIR - 1))
                ob = opool.tile([P, QB], F32, tag="ob", name="ob")
                nc.vector.tensor_copy(ob[:], ps_o[:])
                nc.sync.dma_start(
                    y[ss * P:(ss + 1) * P, jh * QB:(jh + 1) * QB], ob[:])

            drip_work = []

            def do_drip(n):
                for _ in range(n):
                    if not drip_work:
                        return
                    item = drip_work.pop(0)
                    if item[0] == "kq":
                        jkq(pdrip, "d", item[1], item[2], item[3])
                    elif item[0] == "v":
                        jv(pdrip, "d", item[1])
                    else:
                        jop(item[1], item[2])

            # deferred softmax normalization: broadcast the raw Z row with
            # a K=1 fp32r ones-matmul, then take the fast reciprocal of the
            # broadcast (same DVE cost as a 1-row reciprocal) and multiply.
            # No GpSimd and no dtype-convert in the chain.
            pending = []

            def norm_stage_b(keep=0):
                while len(pending) > keep:
                    zrow, u_sb, aslc_ab = pending.pop(0)
                    for h in range(2):
                        bc_ps = pdrip.tile([P, QB], F32, tag="d",
                                           name="bc_ps")
                        nc.tensor.matmul(
                            bc_ps[0:D, :],
                            ones_bf[0:1, 0:D],
                            zrow[:, h * QB:(h + 1) * QB],
                            start=True, stop=True)
                        bcr = spool.tile([D, QB], F32, tag=f"bcr{h}",
                                         name="bcr")
                        nc.vector.reciprocal_approx_fast(
                            bcr[:], bc_ps[0:D, :])
                        nc.vector.tensor_mul(
                            aslc_ab[h],
                            u_sb[:, h * QB:(h + 1) * QB],
                            bcr[:])

            for qi in range(NQ):
                qs = slice(qi * QB, (qi + 1) * QB)
                for pair in range(NPAIR):
                    it = qi * NPAIR + pair
                    # ---- job pushes whose deps are already met ------
                    if it == 0:
                        drip_work.extend(
                            [("kq", "k", 0, 1), ("v", 4), ("v", 5),
                             ("kq", "k", 0, 2), ("v", 6), ("v", 7),
                             ("v", 8), ("kq", "k", 0, 3)]
                            + [("v", ss) for ss in range(9, NSS)]
                            + [("kq", "k", 1, s) for s in range(NQ)]
                            + [("kq", "q", 1, 0)])
                    elif qi == 0 and pair < NPAIR - 1:
                        drip_work.extend(
                            [("kq", "k", pair + 1, s) for s in range(NQ)]
                            + [("kq", "q", pair + 1, 0)])
                    elif qi == 0:
                        drip_work.extend(
                            [("kq", "q", p, 1) for p in range(NPAIR)])
                    elif pair == 0 and qi < NQ - 1:
                        drip_work.extend(
                            [("kq", "q", p, qi + 1) for p in range(NPAIR)])

                    pv = ppv.tile([D + 1, 2 * QB], F32, tag="pv",
                                  name="pv")

                    def emit_pv(ks, pt):
                        for h in range(2):
                            nc.tensor.matmul(
                                pv[:, h * QB:(h + 1) * QB],
                                V2[:, ks, 2 * pair + h, :],
                                pt[:, h * QB:(h + 1) * QB],
                                start=(ks == 0), stop=(ks == NK - 1))

                    # PV is deferred one ks step so the next chunk's scores
                    # matmuls never sit behind a PV that waits on exp
                    prev_pv = None
                    for ks in range(NK):
                        sc = psc.tile([P, 2 * QB], F32, tag="sc", name="sc")
                        for h in range(2):
                            nc.tensor.matmul(
                                sc[:, h * QB:(h + 1) * QB],
                                KT[h * D:(h + 1) * D, pair,
                                   ks * P:(ks + 1) * P],
                                QT[h * D:(h + 1) * D, pair, qs],
                                start=True, stop=True)
                        pt = ptpool.tile([P, 2 * QB], BF16, tag="pt",
                                         name="pt")
                        nc.scalar.activation(pt[:], sc[:], EXP, scale=0.125)
                        if prev_pv is not None:
                            emit_pv(*prev_pv)
                        prev_pv = (ks, pt)
                        if it == 0 or ks % 2 == 0:
                            do_drip(1)
                    emit_pv(*prev_pv)
                    last2 = it >= NQ * NPAIR - 2
                    norm_stage_b(keep=0 if last2 else 1)
                    # stage A: copy out the Z row and the unnormalized
                    # values (frees the PSUM accumulator)
                    zrow = spool.tile([1, 2 * QB], BF16,
                                      tag="zrow", name="zrow")
                    nc.vector.tensor_copy(zrow[:], pv[D:D + 1, :])
                    u_sb = spool.tile([D, 2 * QB], BF16, tag="u", name="u")
                    nc.vector.tensor_copy(u_sb[:], pv[0:D, :])
                    pending.append(
                        (zrow, u_sb,
                         [AOT[h * D:(h + 1) * D, pair, qs]
                          for h in range(2)]))
                    # o_proj of block qi-1 becomes legal once the pending
                    # chain has flushed its pair-3 entry (two iterations)
                    if qi > 0 and pair == 1:
                        drip_work.extend(
                            [("op", ss, jh)
                             for ss in range((qi - 1) * NQ, qi * NQ)
                             for jh in range(2)])
                    do_drip(4)
            norm_stage_b()
            do_drip(len(drip_work))
            # tail o_proj: the scores pool is idle now -- alternate between
            # it and the drip pool for a 4-buffer pipeline
            for i, (ss, jh) in enumerate(
                    [(ss, jh) for ss in range((NQ - 1) * NQ, NSS)
                     for jh in range(2)]):
                if i % 2 == 0:
                    jop(ss, jh, psc.tile([P, 2 * QB], F32, tag="sc",
                                         name="ps_o")[:, 0:QB])
                else:
                    jop(ss, jh)

        main_cm.__exit__(None, None, None)

    nc.compile()
    return nc


def prep_in_maps(x, Wq, bq, Wk, bk, Wv, bv, Wo, bo, head_mask):
    """Host-side shard + layout prep. Returns per-core input dicts."""
    xT = [np.ascontiguousarray(np.asarray(x[b]).T).astype(NP_BF16)
          for b in range(B)]
    per_group: dict = {}
    in_maps = []
    for c in range(NCORES):
        b, g = c // 2, c % 2
        rows = slice(g * O, (g + 1) * O)
        mask = np.repeat(np.asarray(head_mask[8 * g:8 * (g + 1)],
                                    dtype=np.float32), D)
        if g not in per_group:
            per_group[g] = {
                "wqT": np.ascontiguousarray(
                    np.asarray(Wq)[rows, :].T).astype(NP_BF16),
                "wkT": np.ascontiguousarray(
                    np.asarray(Wk)[rows, :].T).astype(NP_BF16),
                "wvT": np.ascontiguousarray(
                    np.asarray(Wv)[rows, :].T).astype(NP_BF16),
                "woT": np.ascontiguousarray(
                    np.asarray(Wo)[:, rows].T * mask[:, None]
                ).astype(NP_BF16),
                "bq": np.ascontiguousarray(
                    np.asarray(bq)[rows].reshape(NPAIR, P).T,
                    dtype=np.float32),
                "bk": np.ascontiguousarray(
                    np.asarray(bk)[rows].reshape(NPAIR, P).T,
                    dtype=np.float32),
                "bv": np.asarray(bv, dtype=np.float32)[rows].reshape(1, O),
            }
        m = dict(per_group[g])
        m["xT"] = xT[b]
        in_maps.append(m)
    return in_maps


def run(in_maps, trace=False):
    if "nc" not in _CACHE:
        _CACHE["nc"] = build_nc()
    return run_bass_kernel_spmd(_CACHE["nc"], in_maps, list(range(NCORES)),
                                trace=trace)


def kernel(x, Wq, bq, Wk, bk, Wv, bv, Wo, bo, head_mask):
    in_maps = prep_in_maps(x, Wq, bq, Wk, bk, Wv, bv, Wo, bo, head_mask)
    res = run(in_maps).results
    bo = np.asarray(bo, dtype=np.float32)
    out = np.empty((B, S, HID), dtype=np.float32)
    for b in range(B):
        out[b] = res[2 * b]["y"] + res[2 * b + 1]["y"] + bo
    return out
